# revision 51
# baseline (speedup 1.0000x reference)
"""CRF forward (logsumexp over paths) loss kernel for Trainium2, 8 NeuronCores.

Time-parallel chunked algorithm, v3 (stacked quadrants + pipelined halves)
--------------------------------------------------------------------------
The linear-space recurrence  w_t = (ETs^T w_{t-1}) * e_t  (ETs = exp(trans-D),
e_t = exp(emit_t)) forgets its initial condition at the Birkhoff contraction
rate, so the T=512 serial chain is cut into S=32 chunks of P=16 steps run
concurrently, each seeded from the raw emission M=1 steps early; the unknown
per-chunk log-magnitude offset is recovered by matching log-colsums (Z) with
the previous chunk at the shared boundary step.

Both 16-chunk pair-groups are STACKED on the 128 SBUF partitions (pair A on
0:64, pair B on 64:128); each step's two 64x64 transition matmuls run
CONCURRENTLY on PE quadrants (0,0)/(64,64).  The 1024 state columns are split
into X/Y halves forming two independent serial chains that ping-pong: the DVE
multiplies half X while the PE runs half Y's matmuls (GpSimd takes the Y
multiplies), hiding the elementwise time.

Z colsums are only USED at rows {0,15,16} (boundary stitching) plus ONE
data-dependent select row per batch element.  Stitch rows: 6 scatter matmuls
(slots 0/15/16) accumulate into a f32 PSUM tile [6,1024].  Select: each batch
element gets a DEDICATED 65th-per-b state column in a tiny parallel stream
[128,64] that replicates its select-chunk's column (host stages identical
emissions on both partition halves); a per-step [128->34] scatter matmul
harvests that stream's colsums into PSUM [34,64], and a host-built one-hot
row mask picks Z(r*_b) -- fully static instruction stream, no indirection.
All exp()s are host-side; select + stitch + batch-sum collapse into mask
dots; DELTA*tau is added on host after gather.  Batch 512 = 8 cores x 64.
"""

import os
import sys

for _p in ("/opt/trn_rl_repo", "/root/.axon_site/_ro/trn_rl_repo"):
    if os.path.isdir(_p) and _p not in sys.path:
        sys.path.insert(0, _p)

from contextlib import ExitStack

import numpy as np

import concourse.bass as bass
import concourse.mybir as mybir
import concourse.tile as tile
from concourse.bass_utils import run_bass_kernel_spmd

# Walrus in this container rejects instructions with >1 sync-wait; split the
# extras onto preceding same-engine no-ops (queues are in-order, so identical
# semantics).
_ORIG_COMMIT = tile.TileContext._commit_instruction


def _single_wait_commit(self, inst, lazy_reg_writes=True):
    si = getattr(inst, "sync_info", None)
    if (
        si is not None
        and si.on_wait
        and len(si.on_wait) > 1
        and inst.engine != mybir.EngineType.Unassigned
    ):
        waits = list(si.on_wait)
        eng = self.nc.engines[inst.engine]
        for w in waits[:-1]:
            n = eng.nop(nofuse=True)
            n.ins.sync_info = mybir.SyncInfo(on_wait=[w], on_update=[])
        inst.sync_info = mybir.SyncInfo(
            on_wait=[waits[-1]], on_update=list(si.on_update or [])
        )
    _ORIG_COMMIT(self, inst, lazy_reg_writes)


tile.TileContext._commit_instruction = _single_wait_commit

T, B, K = 512, 512, 64
NCORES = 8
BSH = B // NCORES      # 64 batch per core
P = 16                 # real steps per chunk
M = 1                  # burn-in steps
S = T // P             # 32 chunks
GP = 16                # chunks per pair-group
PC = GP * BSH          # 1024 columns per pair-group
HC = PC // 2           # 512 columns per matmul (one PSUM bank)
NR = P + 1             # 17 slots (local steps 0..16)
NZS = 2 * NR           # 34 select-harvest rows
DELTA = 4.0            # per-step log-space offset folded into ETs
NWARM = 3              # PE p-state warm-up matmuls
F32 = mybir.dt.float32
BF16 = mybir.dt.bfloat16
FP8 = mybir.dt.float8e4  # emissions only (DVE multiply operand, never PE)
MULT = mybir.AluOpType.mult
ADD = mybir.AluOpType.add
AF = mybir.ActivationFunctionType
AX = mybir.AxisListType.X


def _t_start(c):
    return 0 if c == 0 else c * P - M


def _build_crf_nc() -> bass.Bass:
    nc = bass.Bass(trn_type="TRN2", target_bir_lowering=False, debug=False)

    w0_d = nc.dram_tensor("wring0", [128, PC], BF16, kind="ExternalInput").ap()
    emt_d = nc.dram_tensor("emt", [128, P * PC], FP8, kind="ExternalInput").ap()
    ws0_d = nc.dram_tensor("wsel0", [128, BSH], BF16, kind="ExternalInput").ap()
    esl_d = nc.dram_tensor("esel", [128, P * BSH], FP8,
                           kind="ExternalInput").ap()
    ets_d = nc.dram_tensor("ets", [128, K], BF16, kind="ExternalInput").ap()
    stw_d = nc.dram_tensor("stw", [128, 10], BF16, kind="ExternalInput").ap()
    zw_d = nc.dram_tensor("zw", [128, NR * NZS], BF16,
                          kind="ExternalInput").ap()
    stmA_d = nc.dram_tensor("stmaskA", [4, PC], F32, kind="ExternalInput").ap()
    stmB_d = nc.dram_tensor("stmaskB", [2, PC], F32, kind="ExternalInput").ap()
    sm_d = nc.dram_tensor("smask", [NZS, BSH], F32, kind="ExternalInput").ap()
    out_d = nc.dram_tensor("out_sum", [1, 1], F32, kind="ExternalOutput").ap()

    with tile.TileContext(nc) as tc:
        with ExitStack() as ctx:
            _crf_body(ctx, tc, w0_d, emt_d, ws0_d, esl_d, ets_d, stw_d, zw_d,
                      stmA_d, stmB_d, sm_d, out_d)
    _split_remaining_multiwaits(nc)
    return nc


def _split_remaining_multiwaits(nc):
    for blk in nc.m.functions[0].blocks:
        il = blk.instructions
        idx = 0
        while idx < len(il):
            inst = il[idx]
            si = inst.sync_info
            if si is not None and si.on_wait and len(si.on_wait) > 1:
                waits = list(si.on_wait)
                for j, w in enumerate(waits[:-1]):
                    n = mybir.InstNoOp(
                        name=f"I-swx-{inst.name}-{j}", ins=[], outs=[]
                    )
                    n.engine = inst.engine
                    n.sync_info = mybir.SyncInfo(on_wait=[w], on_update=[])
                    nc.register_instruction(n, overwrite=True)
                    il.insert(idx, n)
                    idx += 1
                inst.sync_info = mybir.SyncInfo(
                    on_wait=[waits[-1]], on_update=list(si.on_update or [])
                )
            idx += 1


def _crf_body(ctx, tc, w0_d, emt_d, ws0_d, esl_d, ets_d, stw_d, zw_d,
              stmA_d, stmB_d, sm_d, out_d):
    nc = tc.nc

    ets = nc.alloc_sbuf_tensor("ets_s", [128, K], BF16).ap()
    stw = nc.alloc_sbuf_tensor("stw_s", [128, 10], BF16).ap()
    zw = nc.alloc_sbuf_tensor("zw_s", [128, NR * NZS], BF16).ap()
    stmA = nc.alloc_sbuf_tensor("stmA_s", [4, PC], F32).ap()
    stmB = nc.alloc_sbuf_tensor("stmB_s", [2, PC], F32).ap()
    smk = nc.alloc_sbuf_tensor("smk_s", [NZS, BSH], F32).ap()
    wring = nc.alloc_sbuf_tensor("wring", [128, 4 * PC], BF16).ap()
    eexp = nc.alloc_sbuf_tensor("eexp", [128, P * PC], FP8).ap()
    wsel = nc.alloc_sbuf_tensor("wsel", [128, 2 * BSH], BF16).ap()
    esel = nc.alloc_sbuf_tensor("esel_s", [128, P * BSH], FP8).ap()
    lnstA = nc.alloc_sbuf_tensor("lnstA", [4, PC], F32).ap()
    scrA = nc.alloc_sbuf_tensor("scrA", [4, PC], F32).ap()
    lnstB = nc.alloc_sbuf_tensor("lnstB", [2, PC], F32).ap()
    scrB = nc.alloc_sbuf_tensor("scrB", [2, PC], F32).ap()
    lnsel = nc.alloc_sbuf_tensor("lnsel", [NZS, BSH], F32).ap()
    scrS = nc.alloc_sbuf_tensor("scrS", [NZS, BSH], F32).ap()
    redA = [nc.alloc_sbuf_tensor(f"redA{h}", [4, 1], F32).ap()
            for h in range(2)]
    redB = [nc.alloc_sbuf_tensor(f"redB{h}", [2, 1], F32).ap()
            for h in range(2)]
    redS = nc.alloc_sbuf_tensor("redS", [NZS, 1], F32).ap()
    dum4 = nc.alloc_sbuf_tensor("dum4", [4, 1], F32).ap()
    dum2 = nc.alloc_sbuf_tensor("dum2", [2, 1], F32).ap()
    dumS = nc.alloc_sbuf_tensor("dumS", [NZS, 1], F32).ap()
    ones = nc.alloc_sbuf_tensor("ones_s", [NZS, 1], F32).ap()
    osb = nc.alloc_sbuf_tensor("osb", [1, 1], F32).ap()
    garb = nc.alloc_sbuf_tensor("garb", [K, HC], BF16).ap()
    dsrc = nc.alloc_sbuf_tensor("dsrc", [1, 2], F32).ap()
    dscr = nc.alloc_sbuf_tensor("dscr", [1, 2], F32).ap()

    # bufs=1: the u-tile WAR (next step's matmuls vs this step's multiply
    # read) is already implied by the serial recurrence through wring
    upool = ctx.enter_context(tc.tile_pool(name="upool", bufs=1, space="PSUM"))
    spool = ctx.enter_context(tc.tile_pool(name="spool", bufs=1, space="PSUM"))

    nc.gpsimd.memset(ones[:, :], 1.0)
    nc.gpsimd.memset(garb[:, :], 0.0)
    nc.gpsimd.memset(dsrc[:, :], 1.0)

    # ---- DMA triggers across all three DGE queues (gpsimd/sync/scalar) ----
    def etrig(eng, s0, ns):  # emission slices s0..s0+ns-1 in one transfer
        eng.dma_start(
            eexp[:, (s0 - 1) * PC : (s0 - 1 + ns) * PC],
            emt_d[:, (s0 - 1) * PC : (s0 - 1 + ns) * PC],
        )

    # need-ordered: per-queue transfers are serial, queues share the HW DMA
    # engines, so only the immediately-needed blocks go first on each queue
    nc.gpsimd.dma_start(wring[:, 0:HC], w0_d[:, 0:HC])
    nc.scalar.dma_start(wring[:, HC:PC], w0_d[:, HC:PC])
    nc.sync.dma_start(ets[:], ets_d)
    etrig(nc.sync, 1, 1)
    etrig(nc.sync, 2, 1)
    nc.gpsimd.dma_start(wsel[:, 0:BSH], ws0_d)
    etrig(nc.scalar, 3, 1)
    nc.gpsimd.dma_start(esel[:], esl_d)
    nc.gpsimd.dma_start(stw[:], stw_d)
    nc.gpsimd.dma_start(zw[:], zw_d)
    etrig(nc.sync, 4, 1)
    etrig(nc.scalar, 5, 1)
    etrig(nc.sync, 6, 1)
    etrig(nc.gpsimd, 7, 2)
    etrig(nc.scalar, 9, 2)
    etrig(nc.sync, 11, 2)
    etrig(nc.gpsimd, 13, 2)
    etrig(nc.scalar, 15, 2)
    nc.sync.dma_start(stmA[:], stmA_d)
    nc.sync.dma_start(stmB[:], stmB_d)
    nc.sync.dma_start(smk[:], sm_d)
    nc.scalar.activation(dscr[:], dsrc[:], AF.Ln)  # act-table preload

    # ---- PE p-state warm-up during the DMA wait ----
    wu = upool.tile([128, HC], F32, tag="ux")
    for _ in range(NWARM):
        nc.tensor.matmul(wu[0:K, :], garb[:, 0:K], garb[:], start=True,
                         stop=True)

    zstA = spool.tile([4, PC], F32, tag="zstA")  # stitch slots 0, 15
    zstB = spool.tile([2, PC], F32, tag="zstB")  # stitch slot 16
    zsa = spool.tile([NZS, BSH], F32, tag="zsa")

    def stitch(i, slot):
        # accumulate Z(stitch slot) into rows 2i (pair A) / 2i+1 (pair B)
        dst, wsl = (zstA, stw[:, 4 * i : 4 * i + 4]) if i < 2 else (
            zstB, stw[:, 8:10])
        for h in range(2):
            nc.tensor.matmul(
                dst[:, h * HC : (h + 1) * HC],
                wsl,
                wring[:, slot * PC + h * HC : slot * PC + (h + 1) * HC],
                start=(i != 1),
                stop=(i != 0),
                skip_group_check=True,
            )

    def sel_harvest(s, slot):
        # zsa rows 2s/2s+1 += colsums of the select stream at local step s.
        # Contracts only partitions 0:64 (the stream is duplicated on both
        # halves) so it runs on the (0,0) PE quadrant, concurrent with the
        # (64,64) chain matmuls.
        nc.tensor.matmul(
            zsa[:, :],
            zw[0:K, s * NZS : (s + 1) * NZS],
            wsel[0:K, slot * BSH : (slot + 1) * BSH],
            start=(s == 0),
            stop=(s == P),
            skip_group_check=True,
        )

    # ---- chain: two ping-ponging column-half streams + select stream ----
    for s in range(1, P + 1):
        prev, cur = (s - 1) % 4, s % 4
        sprev, scur = (s - 1) % 2, s % 2
        for hx in range(2):
            u = upool.tile([128, HC], F32, tag=("ux", "uy")[hx])
            co = hx * HC
            for pr in (0, 1):
                nc.tensor.matmul(
                    u[pr * K : (pr + 1) * K, :],
                    ets[pr * K : (pr + 1) * K, :],
                    wring[
                        pr * K : (pr + 1) * K,
                        prev * PC + co : prev * PC + co + HC,
                    ],
                    start=True,
                    stop=True,
                )
            nc.vector.tensor_tensor(
                wring[:, cur * PC + co : cur * PC + co + HC],
                u[:, :],
                eexp[:, (s - 1) * PC + co : (s - 1) * PC + co + HC],
                op=MULT,
            )
        if s == 1:
            # emitted here (not before the loop) so the PE queue is not
            # head-of-line blocked on the zw/wsel0 DMAs before step 1
            sel_harvest(0, 0)
        us = spool.tile([128, BSH], F32, tag="usel")
        for pr in (0, 1):
            nc.tensor.matmul(
                us[pr * K : (pr + 1) * K, :],
                ets[pr * K : (pr + 1) * K, :],
                wsel[pr * K : (pr + 1) * K, sprev * BSH : (sprev + 1) * BSH],
                start=True,
                stop=True,
            )
        nc.vector.tensor_tensor(
            wsel[:, scur * BSH : (scur + 1) * BSH],
            us[:, :],
            esel[:, (s - 1) * BSH : s * BSH],
            op=MULT,
        )
        sel_harvest(s, scur)
        if s == 1:
            stitch(0, 0)  # Z(0) from the DMA'd init slot
        elif s == P - 1:
            stitch(1, (P - 1) % 4)
            # slots 0+15 combine hides under step 16: Ln + accum on Scalar,
            # mask-multiply on GpSimd (both idle; SBUF-only so GP is legal)
            for h in range(2):
                cs = slice(h * HC, (h + 1) * HC)
                nc.scalar.activation(lnstA[:, cs], zstA[:, cs], AF.Ln)
                nc.gpsimd.tensor_tensor(
                    scrA[:, cs], lnstA[:, cs], stmA[:, cs], op=MULT
                )
                nc.scalar.activation(
                    dum4.broadcast_to(scrA[:, cs].shape), scrA[:, cs],
                    AF.Identity, accum_out=redA[h][:],
                )
    stitch(2, P % 4)

    # ---- combine: ln, mask dots, partition-sum ----
    # column-halved pipeline: ACT does Ln h0 then h1; DVE multiplies each as
    # it lands; GpSimd (idle once its DMA drain clears) does the reduces.
    for h in range(2):
        cs = slice(h * HC, (h + 1) * HC)
        nc.scalar.activation(lnstB[:, cs], zstB[:, cs], AF.Ln)
        nc.vector.tensor_tensor(scrB[:, cs], lnstB[:, cs], stmB[:, cs],
                                op=MULT)
        nc.scalar.activation(
            dum2.broadcast_to(scrB[:, cs].shape), scrB[:, cs], AF.Identity,
            accum_out=redB[h][:],
        )
    nc.scalar.activation(lnsel[:], zsa[:], AF.Ln)
    nc.vector.tensor_tensor(scrS[:], lnsel[:], smk[:], op=MULT)
    nc.scalar.activation(
        dumS.broadcast_to(scrS[:].shape), scrS[:], AF.Identity,
        accum_out=redS[:],
    )
    acc = zstA[0:1, 0:1]
    nc.tensor.matmul(acc, redA[0][:], ones[0:4, :], start=True, stop=False,
                     skip_group_check=True)
    nc.tensor.matmul(acc, redA[1][:], ones[0:4, :], start=False, stop=False,
                     skip_group_check=True)
    nc.tensor.matmul(acc, redB[0][:], ones[0:2, :], start=False, stop=False,
                     skip_group_check=True)
    nc.tensor.matmul(acc, redB[1][:], ones[0:2, :], start=False, stop=False,
                     skip_group_check=True)
    nc.tensor.matmul(acc, redS[:], ones[:, :], start=False, stop=True,
                     skip_group_check=True)
    nc.scalar.copy(osb[:], acc)
    nc.sync.dma_start(out_d, osb[:])


_NC_CACHE = None


def _get_nc():
    global _NC_CACHE
    if _NC_CACHE is None:
        _NC_CACHE = _build_crf_nc()
    return _NC_CACHE


def _make_in_maps(np_inputs):
    import ml_dtypes

    BF = ml_dtypes.bfloat16
    F8 = ml_dtypes.float8_e4m3fn
    emits = np.asarray(np_inputs["emits"], dtype=np.float32)
    mask = np.asarray(np_inputs["mask"])
    transitions = np.asarray(np_inputs["transitions"], dtype=np.float32)
    alpha_0 = np.asarray(np_inputs["alpha_0"], dtype=np.float32)
    tau = mask.argmax(0).astype(np.int64)  # [B]

    exp_emits = np.exp(emits)
    expal = np.exp(alpha_0.reshape(K))
    ets_blk = np.tile(np.exp(transitions - DELTA), (2, 1)).astype(BF)

    # zstA scatter: slot0 -> rows 0/1 (cols 0/1), slot15 -> rows 2/3
    # (cols 6/7); zstB: slot16 -> rows 0/1 (cols 8/9)
    stw_blk = np.zeros((128, 10), dtype=np.float32)
    for base, row in ((0, 0), (4, 2), (8, 0)):
        stw_blk[0:K, base + row] = 1.0
        stw_blk[K:128, base + row + 1] = 1.0
    stw_blk = stw_blk.astype(BF)

    # A-half-only scatter (the select stream is duplicated on both halves);
    # odd rows get the same colsum so no zbuf entry is ln(0)
    zw_blk = np.zeros((128, NR * NZS), dtype=np.float32)
    for s in range(NR):
        zw_blk[0:K, s * NZS + 2 * s] = 1.0
        zw_blk[0:K, s * NZS + 2 * s + 1] = 1.0
    zw_blk = zw_blk.astype(BF)

    ts = np.array(
        [[_t_start(c) + s for c in range(S)] for s in range(P + 1)]
    )

    in_maps = []
    for cix in range(NCORES):
        sl = slice(cix * BSH, (cix + 1) * BSH)
        eT = exp_emits[:, sl, :].transpose(0, 2, 1)  # [T, K, 64]
        blk = (
            eT[ts]
            .reshape(P + 1, 2, GP, K, BSH)
            .transpose(0, 1, 3, 2, 4)
            .reshape(P + 1, 128, PC)
            .copy()
        )
        blk[0, 0:K, 0:BSH] *= expal[:, None]
        # emissions ride in fp8e4 (multiply operand only); clip away the
        # e4m3fn NaN-above-448 and flush-to-zero tails
        emt8 = np.clip(blk[1:], 0.002, 440.0).astype(F8)

        tau_s = tau[sl]
        cb_s = tau_s // P
        # select stream: per-b replica of its select chunk's column, same
        # data on BOTH partition halves (keeps every colsum positive)
        selblk = np.empty((P + 1, K, BSH), dtype=np.float32)
        for bi in range(BSH):
            t0 = _t_start(int(cb_s[bi]))
            selblk[:, :, bi] = eT[t0 : t0 + P + 1, :, bi]
            if cb_s[bi] == 0:
                selblk[0, :, bi] *= expal
        selblk = np.tile(selblk, (1, 2, 1))  # [17, 128, 64]

        stm = np.zeros((6, PC), dtype=np.float32)
        smw = np.zeros((NZS, BSH), dtype=np.float32)
        for bi in range(BSH):
            tb = int(tau_s[bi])
            cb = tb // P
            rstar = tb if cb == 0 else tb % P + 1
            smw[2 * rstar, bi] += 1.0
            for j in range(1, cb + 1):
                if j == 1:
                    stm[2, bi] += 1.0  # chunk 0 provider: slot 15, pair A
                else:
                    stm[4 + (j - 1) // GP, ((j - 1) % GP) * BSH + bi] += 1.0
                stm[0 + j // GP, (j % GP) * BSH + bi] -= 1.0

        in_maps.append(
            {
                "wring0": blk[0].astype(BF),
                "emt": np.ascontiguousarray(
                    emt8.transpose(1, 0, 2)
                ).reshape(128, P * PC),
                "wsel0": selblk[0].astype(BF),
                "esel": np.ascontiguousarray(
                    np.clip(selblk[1:], 0.002, 440.0)
                    .astype(F8).transpose(1, 0, 2)
                ).reshape(128, P * BSH),
                "ets": ets_blk,
                "stw": stw_blk,
                "zw": zw_blk,
                "stmaskA": stm[0:4], "stmaskB": stm[4:6],
                "smask": smw,
            }
        )
    return in_maps


def kernel(emits, mask, transitions, alpha_0):
    nc = _get_nc()
    in_maps = _make_in_maps(
        {"emits": emits, "mask": mask, "transitions": transitions,
         "alpha_0": alpha_0}
    )
    res = run_bass_kernel_spmd(nc, in_maps, core_ids=list(range(NCORES)))
    tau = np.asarray(mask).argmax(0).astype(np.int64)
    total = np.float64(DELTA) * np.float64(tau.sum())
    for r in res.results:
        total += np.asarray(r["out_sum"], dtype=np.float64).sum()
    return np.float32(total)
